# revision 16
# baseline (speedup 1.0000x reference)
"""Distributed brute-force kNN retrieval (cosine similarity) on 8 Trainium2 cores.

Strategy (per spec sharding hint, adapted):
  - Shard the feature bank along N across 8 cores (62500 rows each).
  - Host pre-transposes each shard to a group-contiguous fp8 e4m3 layout
    [128, (group, k, cols)] so each DMA group is one fully-contiguous run per
    partition (best descriptor efficiency; ~415 GB/s sustained).
  - Each core computes raw dot products q @ f_shard.T with fp8 matmuls
    (fp32 PSUM accumulation). Feature chunks are processed two-at-a-time via
    PE column tiling (tile_position=(0,64)): chunk A lands on PSUM partitions
    0-63, chunk B on 64-127. The k-steps of the two halves are interleaved
    (A0 B0 A1 B1 ...) so the two column-tile streams run concurrently on the
    PE array (~2x matmul throughput vs sequential chains).
  - DMA groups are small (5 chunks = 1.92 MB) with a deep tile pool so many
    transfers stay outstanding (keeps SDMA queues fed at max rate) and the PE
    never idles long enough for HAM to re-throttle the clock. First/last
    groups are tapered (2,3 / 3,2 chunks) for a fast start and a short tail.
  - The DVE Max8/MaxIndex instructions extract the top-8 candidates per
    query-row per similarity block. The odd 125th chunk is covered by an
    overlapping final pair (chunks 123,124); the host dedups.
  - Host maps candidates to global row indices, rescores them exactly in fp32
    (normalized cosine similarity, same math as the reference), does the final
    top-k reduction, and gathers the data segments.

The candidate margin (top-8 of every <=1000-col half-block when only the
global top-5 is needed) makes the device pass insensitive to fp8 rounding: a
true top-5 entry would have to be pushed below rank 8 *within its own block*
by fp8 dot-noise (sigma ~0.7) across gaps that total many sigma. The exact
host rescore then removes all remaining matmul error.
"""

import os
import sys

import numpy as np

import concourse.bacc as bacc
import concourse.mybir as mybir
from concourse.tile import TileContext
from concourse.bass_utils import run_bass_kernel_spmd


def _ensure_ntff_hook():
    """run_bass_kernel_spmd(trace) under axon imports antenv.axon_hooks,
    which this container image lacks. Provide the shim (profiling works) or
    disable tracing so a stray BASS_TRACE env var cannot crash the run."""
    try:
        import antenv.axon_hooks  # noqa: F401
        return
    except ImportError:
        pass
    try:
        import types
        from trn_agent_boot.trn_boot import _ntff_profile_via_ctypes
        hook = _ntff_profile_via_ctypes("/opt/axon/libaxon_pjrt.so")
        mod = types.ModuleType("antenv.axon_hooks")
        mod.get_axon_ntff_profile_hook = lambda: hook
        mod.set_axon_ntff_profile_hook = lambda h: None
        sys.modules["antenv.axon_hooks"] = mod
        import antenv
        antenv.axon_hooks = mod
    except Exception:
        os.environ["BASS_NEVER_TRACE"] = "1"

# Problem geometry (hardcoded per spec).
B = 64             # queries
D = 768            # feature dim
N = 500000         # feature rows
NCORES = 8
NSH = N // NCORES  # 62500 rows per core
KC = D // 128      # 6 contraction chunks of 128
CHUNK = 500        # matmul moving free dim (one PSUM bank)
NCHUNKS = NSH // CHUNK           # 125 chunks
NPAIRS = 63                      # pairs (2j, 2j+1); pair 62 = (123, 124) overlaps
BLOCKS = [2] * 30 + [1] * 3      # pairs per Max8 block (sum = 63); 1-pair
                                 # final blocks keep the DVE tail short
NBLOCKS = len(BLOCKS)
BLOCK_BASE = [sum(BLOCKS[:i]) for i in range(NBLOCKS)]
TOPB = 8                         # Max8 output width per block
MAXBP = max(BLOCKS)

# DMA groups (in chunks): one group per matmul pair (768 KB) so the PE's
# data dependency is fine-grained — it never waits more than one small
# transfer, keeping its idle gaps well under the ~3.4us HAM re-throttle
# window. Each group is one contiguous [128, 6*L*500] fp8 run in DRAM.
GROUP_SIZES = [1, 1] + [2] * 61 + [1]
assert sum(GROUP_SIZES) == NCHUNKS
GROUP_STARTS = [sum(GROUP_SIZES[:i]) for i in range(len(GROUP_SIZES))]
NGROUPS = len(GROUP_SIZES)
GROUP_OFFS = [KC * CHUNK * s for s in GROUP_STARTS]  # per-partition elem offs
LMAX = max(GROUP_SIZES)
FBUFS = 22                       # f-group tile pool depth (deep SDMA queues)
NWARM = 5                        # PE warm-up matmuls (HAM K=8/8 before stream)
# Output DMA slices (by Max8 block) so only the last sliver is on the tail.
OUT_SLICES = [(0, 12), (12, 22), (22, 30), (30, 33)]

_CHUNK_GROUP = {}
for _g, (_s, _L) in enumerate(zip(GROUP_STARTS, GROUP_SIZES)):
    for _lc in range(_L):
        _CHUNK_GROUP[_s + _lc] = (_g, _lc)

_COMPILED = None
LAST_RESULTS = None  # test harness introspection


def _pair_chunks(j):
    return (2 * j, 2 * j + 1) if j < 62 else (123, 124)


def _build():
    nc = bacc.Bacc("TRN2", target_bir_lowering=False, debug=False)
    # q is packed host-side to [128, KC*B] (partition-major) so its DMA is
    # 128 contiguous descriptors, not 768x 64-byte ones (which took ~10us
    # at the head of the FIFO ring, stalling the whole feature stream).
    qT = nc.declare_dram_parameter("qT", [128, KC * B], mybir.dt.float8e4, isOutput=False)
    fT = nc.declare_dram_parameter(
        "fT", [128, KC * NSH], mybir.dt.float8e4, isOutput=False
    )
    out_vals = nc.declare_dram_parameter(
        "vals", [128, NBLOCKS * TOPB], mybir.dt.float32, isOutput=True
    )
    out_idx = nc.declare_dram_parameter(
        "idx", [128, NBLOCKS * TOPB], mybir.dt.uint32, isOutput=True
    )

    fT_ap = fT.ap()

    with TileContext(nc) as tc:
        with (
            tc.tile_pool(name="qpool", bufs=1) as qpool,
            tc.tile_pool(name="wpool", bufs=1) as wpool,
            tc.tile_pool(name="fpool", bufs=FBUFS) as fpool,
            tc.tile_pool(name="simspool", bufs=5) as simspool,
            tc.tile_pool(name="outpool", bufs=1) as outpool,
            tc.tile_pool(name="psum", bufs=7, space="PSUM") as psump,
            tc.tile_pool(name="wpsum", bufs=1, space="PSUM") as wpsump,
        ):
            q_sb = qpool.tile([128, KC * B], mybir.dt.float8e4)

            # PE warm-up: a burst of throwaway matmuls on a zeroed scratch
            # tile keeps the PE busy through one HAM activity window so the
            # clock gate opens (K=8/8) before the first feature pair lands.
            warm = wpool.tile([128, CHUNK], mybir.dt.float8e4)
            nc.gpsimd.memset(warm[:], 0)
            wps = wpsump.tile([128, CHUNK], mybir.dt.float32)
            for w in range(NWARM):
                nc.tensor.matmul(
                    wps[0:B, :],
                    lhsT=warm[:, :B],
                    rhs=warm[:, :],
                    start=(w == 0),
                    stop=(w == NWARM - 1),
                )

            vals_st = outpool.tile([128, NBLOCKS * TOPB], mybir.dt.float32)
            idx_st = outpool.tile([128, NBLOCKS * TOPB], mybir.dt.uint32)

            g_tiles = {}
            loaded = [0]  # groups loaded so far

            def load_group():
                g = loaded[0]
                L = GROUP_SIZES[g]
                f_sb = fpool.tile([128, KC * LMAX * CHUNK], mybir.dt.float8e4)
                n = KC * L * CHUNK
                nc.sync.dma_start(
                    out=f_sb[:, :n],
                    in_=fT_ap[:, GROUP_OFFS[g]:GROUP_OFFS[g] + n],
                )
                g_tiles[g] = (f_sb, L)
                loaded[0] = g + 1

            def rhs(chunk, k):
                g, lc = _CHUNK_GROUP[chunk]
                f_sb, L = g_tiles[g]
                o = (k * L + lc) * CHUNK
                return f_sb[:, o:o + CHUNK]

            def load_until(c):
                while loaded[0] <= _CHUNK_GROUP[c][0]:
                    load_group()

            # Deep prefetch: keep the SDMA queues full from the start.
            # The first two (single-chunk) feature groups go ahead of q so
            # the PE's first real work is ready as early as possible.
            load_group()
            load_group()
            nc.sync.dma_start(out=q_sb[:], in_=qT.ap())
            for _ in range(min(FBUFS, NGROUPS) - 2):
                load_group()

            for blk in range(NBLOCKS):
                bpairs = BLOCKS[blk]
                bsize = bpairs * CHUNK
                sims = simspool.tile([128, MAXBP * CHUNK], mybir.dt.float32)
                if bpairs == 2 and BLOCK_BASE[blk] > 0:
                    # k-outer superblock: both pairs of the block share each
                    # q_k weight load; consecutive same-tile matmuls reuse
                    # the stationary operand, halving LDWEIGHTS pressure.
                    pjs = [BLOCK_BASE[blk], BLOCK_BASE[blk] + 1]
                    chs = [_pair_chunks(p) for p in pjs]
                    load_until(chs[1][1])
                    pss = [psump.tile([128, CHUNK], mybir.dt.float32, name="ps")
                           for _ in range(2)]
                    for k in range(KC):
                        for i in range(2):
                            nc.tensor.matmul(
                                pss[i][0:B, :],
                                lhsT=q_sb[:, k * B:(k + 1) * B],
                                rhs=rhs(chs[i][0], k),
                                start=(k == 0),
                                stop=(k == KC - 1),
                            )
                        for i in range(2):
                            nc.tensor.matmul(
                                pss[i][B:2 * B, :],
                                lhsT=q_sb[:, k * B:(k + 1) * B],
                                rhs=rhs(chs[i][1], k),
                                start=(k == 0),
                                stop=(k == KC - 1),
                                tile_position=(0, B),
                            )
                    for i in range(2):
                        nc.scalar.copy(
                            out=sims[:, i * CHUNK:(i + 1) * CHUNK],
                            in_=pss[i][:],
                        )
                    pair_iter = []
                else:
                    pair_iter = list(range(bpairs))
                for j in pair_iter:
                    pj = BLOCK_BASE[blk] + j
                    ca, cb = _pair_chunks(pj)
                    ps = psump.tile([128, CHUNK], mybir.dt.float32)
                    # Interleave the two column-tile halves per k-step so the
                    # A (cols 0-63) and B (cols 64-127) streams overlap on PE.
                    # Pair 0 runs chain-ordered instead so its A half starts
                    # as soon as the first single-chunk group lands.
                    if pj == 0:
                        load_until(ca)
                        for k in range(KC):
                            nc.tensor.matmul(
                                ps[0:B, :],
                                lhsT=q_sb[:, k * B:(k + 1) * B],
                                rhs=rhs(ca, k),
                                start=(k == 0),
                                stop=(k == KC - 1),
                            )
                        load_until(cb)
                        for k in range(KC):
                            nc.tensor.matmul(
                                ps[B:2 * B, :],
                                lhsT=q_sb[:, k * B:(k + 1) * B],
                                rhs=rhs(cb, k),
                                start=(k == 0),
                                stop=(k == KC - 1),
                                tile_position=(0, B),
                            )
                    else:
                        load_until(cb)
                        for k in range(KC):
                            nc.tensor.matmul(
                                ps[0:B, :],
                                lhsT=q_sb[:, k * B:(k + 1) * B],
                                rhs=rhs(ca, k),
                                start=(k == 0),
                                stop=(k == KC - 1),
                            )
                            nc.tensor.matmul(
                                ps[B:2 * B, :],
                                lhsT=q_sb[:, k * B:(k + 1) * B],
                                rhs=rhs(cb, k),
                                start=(k == 0),
                                stop=(k == KC - 1),
                                tile_position=(0, B),
                            )
                    if blk == NBLOCKS - 1:
                        last_ps = ps  # final block reads PSUM directly
                    else:
                        nc.scalar.copy(
                            out=sims[:, j * CHUNK:(j + 1) * CHUNK], in_=ps[:]
                        )
                src_ap = last_ps[:] if blk == NBLOCKS - 1 else sims[:, :bsize]
                nc.vector.max(
                    out=vals_st[:, blk * TOPB:(blk + 1) * TOPB],
                    in_=src_ap,
                )
                nc.vector.max_index(
                    out=idx_st[:, blk * TOPB:(blk + 1) * TOPB],
                    in_max=vals_st[:, blk * TOPB:(blk + 1) * TOPB],
                    in_values=src_ap,
                )
                for lo, hi in OUT_SLICES:
                    if blk == hi - 1:
                        # Scalar-ring DMA keeps the sync ring FIFO for the
                        # feature stream; only the last sliver is tail work.
                        nc.scalar.dma_start(
                            out=out_vals.ap()[:, lo * TOPB:hi * TOPB],
                            in_=vals_st[:, lo * TOPB:hi * TOPB],
                        )
                        nc.scalar.dma_start(
                            out=out_idx.ap()[:, lo * TOPB:hi * TOPB],
                            in_=idx_st[:, lo * TOPB:hi * TOPB],
                        )

    nc.compile()
    return nc


def _get_compiled():
    global _COMPILED
    if _COMPILED is None:
        _COMPILED = _build()
    return _COMPILED


def _pack_shard(shard_T_f8):
    """[768, NSH] fp8 -> [128, KC*NSH] group-contiguous device layout."""
    A3 = shard_T_f8.reshape(KC, 128, NSH)
    parts = []
    for s, L in zip(GROUP_STARTS, GROUP_SIZES):
        blk = A3[:, :, s * CHUNK:(s + L) * CHUNK]      # [KC, 128, L*CHUNK]
        parts.append(np.transpose(blk, (1, 0, 2)).reshape(128, KC * L * CHUNK))
    return np.ascontiguousarray(np.concatenate(parts, axis=1))


def _candidates(idx_arr, val_arr):
    """Map device Max8 outputs (128, NBLOCKS*8) to (feature rows, dot vals).

    Row p < 64 is query p over the first chunk of each pair; row p >= 64 is
    query p-64 over the second chunk. Block b covers pairs starting at
    BLOCK_BASE[b]; a Max8 index i within the block means pair
    BLOCK_BASE[b] + i//CHUNK at position i%CHUNK.
    """
    slot_block = np.repeat(np.arange(NBLOCKS), TOPB)  # (NBLOCKS*TOPB,)
    base = np.array(BLOCK_BASE)[slot_block]
    rows_out, vals_out = [], []
    for q in range(B):
        rows, vals = [], []
        for half in (0, 1):
            i = idx_arr[q + half * B].astype(np.int64)  # (NBLOCKS*TOPB,)
            pair = base + i // CHUNK
            chunk = np.where(pair < 62, 2 * pair + half, 123 + half)
            rows.append(chunk * CHUNK + i % CHUNK)
            vals.append(val_arr[q + half * B])
        rows_out.append(np.concatenate(rows))
        vals_out.append(np.concatenate(vals))
    return np.stack(rows_out), np.stack(vals_out)  # (B, 2*NBLOCKS*TOPB) each


def kernel(query_feature, feature, data, k=5, **kwargs):
    global LAST_RESULTS
    q = np.ascontiguousarray(np.asarray(query_feature, dtype=np.float32))
    f = np.asarray(feature, dtype=np.float32)
    data = np.asarray(data)
    k = int(k)
    assert q.shape == (B, D) and f.shape == (N, D)

    nc = _get_compiled()

    F8 = mybir.dt.np(mybir.dt.float8e4)
    qT = np.ascontiguousarray(
        q.T.astype(F8).reshape(KC, 128, B).transpose(1, 0, 2).reshape(128, KC * B)
    )
    in_maps = []
    for i in range(NCORES):
        fT = _pack_shard(f[i * NSH:(i + 1) * NSH].T.astype(F8))
        in_maps.append({"qT": qT, "fT": fT})

    _ensure_ntff_hook()
    res = run_bass_kernel_spmd(nc, in_maps, core_ids=list(range(NCORES)))
    LAST_RESULTS = res

    all_rows, all_vals = [], []
    for i in range(NCORES):
        rows, vals = _candidates(res.results[i]["idx"], res.results[i]["vals"])
        all_rows.append(i * NSH + rows)
        all_vals.append(vals)
    cand_all = np.concatenate(all_rows, axis=1)  # (B, NCORES*2*NBLOCKS*TOPB)
    vals_all = np.concatenate(all_vals, axis=1)

    # Prefilter by device dot value (fp8 noise sigma ~0.7 on gaps ~30 sigma):
    # keep the top PREK per query, then rescore those exactly.
    PREK = 96
    pre = np.argpartition(-vals_all, PREK, axis=1)[:, :PREK]
    cand = np.take_along_axis(cand_all, pre, axis=1)  # (B, PREK)

    # Exact fp32 rescore of candidates (same math as the reference).
    qn = q / np.linalg.norm(q, axis=1, keepdims=True)
    fc = f[cand]  # (B, C, D)
    fn = fc / np.linalg.norm(fc, axis=2, keepdims=True)
    sims = np.einsum("bd,bcd->bc", qn, fn)  # fp32

    # Final top-k with jax.lax.top_k tie-breaking (value desc, index asc).
    # Chunk 123/124 features can appear twice (overlapping final pair):
    # sort by index, mask duplicate neighbors.
    o = np.argsort(cand, axis=1, kind="stable")
    cand_s = np.take_along_axis(cand, o, axis=1)
    sims_s = np.take_along_axis(sims, o, axis=1)
    dup = np.zeros_like(sims_s, dtype=bool)
    dup[:, 1:] = cand_s[:, 1:] == cand_s[:, :-1]
    sims_s = np.where(dup, -np.inf, sims_s)
    sel = np.argsort(-sims_s, axis=1, kind="stable")[:, :k]
    top_idx = np.take_along_axis(cand_s, sel, axis=1)  # (B, k)

    return data[top_idx]  # (B, k, data_cols), input dtype preserved
